# revision 1
# baseline (speedup 1.0000x reference)
"""ChessBoardAttention Trainium2 kernel.

Math (per chessboard window of the input):
  x: [B=2, C=128, H=256, W=256] f32.  WS=8 chessboard phases.
  window (b, ph, pw) owns tokens (h, w) with h%8==ph, w%8==pw -> N=1024 tokens.
  q = x@Wq.T+bq [N,32]; k = x@Wk.T+bk [N,32]; v = x@Wv.T+bv [N,128]
  out = softmax(q k^T) v ; y = gamma*out + x

Sharding: 16 row-groups (b, ph), 2 per core. Each row-group holds the 8
pw-windows built from rows h==ph (mod 8) of batch b -> x[b,:,ph::8,:]
([128, 32, 256] slab, channel-partitioned). All compute for a window runs
on one core; no collectives.

Per-window on-chip pipeline (channel/token layouts chosen so softmax stats
are per-partition and the attention transpose rides the DMA XBAR):
  x_win  = stride-8 view of the slab: [c=128, t=1024]
  q^T,k^T = W^T.T @ x_win           (PE, bf16)   [32, 1024]
  v      = x_chunk.T @ Wv^T          (PE, bf16)   [m=128, c=128] per 128-token chunk
  S      = q_chunk.T @ k^T           (PE, bf16)   [n=128, m=1024] per n-chunk
  exp    = ACT Exp with accum_out -> Z[n] row sums
  attn   = exp * (gamma/Z[n])        (DVE, per-partition scalar)
  attn^T = DMA XBAR transpose        (SDMA, bf16)
  out^T  = v.T @ attn^T              (PE, accumulate over m-chunks) [c, 1024]
  y      = out^T + gamma*bv + x_win  (DVE scalar_tensor_tensor, in-place into slab)

softmax max-subtraction is dropped: scores are ~N(0, 0.3), exp is safe, and
softmax is shift-invariant so the result matches the reference.
"""

import sys

if "/opt/trn_rl_repo" not in sys.path:
    sys.path.insert(0, "/opt/trn_rl_repo")

from contextlib import ExitStack

import ml_dtypes
import numpy as np

import concourse.bacc as bacc
import concourse.bass as bass
import concourse.mybir as mybir
from concourse import bass_utils
from concourse.tile import TileContext

B, C, H, W = 2, 128, 256, 256
WS = 8
NH, NW = H // WS, W // WS  # 32, 32
N = NH * NW  # 1024 tokens per window
D = C // 4  # 32 q/k channels
NCORES = 8
PAIRS = 2  # (b, ph) row-groups per core
NCH = N // 128  # 8 chunks of 128 tokens
F32 = mybir.dt.float32
BF16 = mybir.dt.bfloat16

TRACE = False
LAST = {}

_CACHE = {}

def _emit(nc: bass.Bass):
    # xs is HOST-PERMUTED window-major: xs[g, c, pw, t] = x[b, c, (t//32)*8+ph, (t%32)*8+pw]
    xs = nc.dram_tensor("xs", [PAIRS, C, WS, N], F32, kind="ExternalInput").ap()
    wq = nc.dram_tensor("wq", [C, D], BF16, kind="ExternalInput").ap()
    wk = nc.dram_tensor("wk", [C, D], BF16, kind="ExternalInput").ap()
    wv = nc.dram_tensor("wv", [C, C], BF16, kind="ExternalInput").ap()
    bq = nc.dram_tensor("bq", [D, 1], F32, kind="ExternalInput").ap()
    bk = nc.dram_tensor("bk", [D, 1], F32, kind="ExternalInput").ap()
    gv = nc.dram_tensor("gv", [C, 1], F32, kind="ExternalInput").ap()  # gamma*bv
    gam = nc.dram_tensor("gam", [C, 1], F32, kind="ExternalInput").ap()  # gamma
    ys = nc.dram_tensor("ys", [PAIRS, C, WS, N], F32, kind="ExternalOutput").ap()

    with ExitStack() as ctx:
        tc = ctx.enter_context(TileContext(nc))
        consts = ctx.enter_context(tc.tile_pool(name="consts", bufs=1))
        xpool = ctx.enter_context(tc.tile_pool(name="xpool", bufs=2))
        xbpool = ctx.enter_context(tc.tile_pool(name="xbpool", bufs=2))
        qkpool = ctx.enter_context(tc.tile_pool(name="qkpool", bufs=2))
        vpool = ctx.enter_context(tc.tile_pool(name="vpool", bufs=2))
        epool = ctx.enter_context(tc.tile_pool(name="epool", bufs=2))
        apool = ctx.enter_context(tc.tile_pool(name="apool", bufs=2))
        # bufs=16: one slot per window -> no slot reuse -> no WAR waits on the
        # ACT exp instructions (TensorScalarPtr/ACTIVATE have few wait slots)
        zpool = ctx.enter_context(tc.tile_pool(name="zpool", bufs=16))
        ps_s = ctx.enter_context(tc.tile_pool(name="ps_s", bufs=2, space="PSUM"))
        ps_o = ctx.enter_context(tc.tile_pool(name="ps_o", bufs=1, space="PSUM"))
        ps_m = ctx.enter_context(tc.tile_pool(name="ps_m", bufs=1, space="PSUM"))

        wq_sb = consts.tile([C, D], BF16)
        nc.sync.dma_start(out=wq_sb, in_=wq)
        wk_sb = consts.tile([C, D], BF16)
        nc.sync.dma_start(out=wk_sb, in_=wk)
        wv_sb = consts.tile([C, C], BF16)
        nc.sync.dma_start(out=wv_sb, in_=wv)
        bq_sb = consts.tile([D, 1], F32)
        nc.sync.dma_start(out=bq_sb, in_=bq)
        bk_sb = consts.tile([D, 1], F32)
        nc.sync.dma_start(out=bk_sb, in_=bk)
        gv_sb = consts.tile([C, 1], F32)
        nc.sync.dma_start(out=gv_sb, in_=gv)
        gam_sb = consts.tile([C, 1], F32)
        nc.sync.dma_start(out=gam_sb, in_=gam)

        # Touch every const once on DVE so later DVE ops (esp. TensorScalarPtr,
        # which walrus limits to ONE sync wait) never carry const-DMA waits.
        scratch = consts.tile([C, 8], F32)
        for i, t in enumerate([wq_sb, wk_sb, wv_sb, bq_sb, bk_sb, gv_sb, gam_sb]):
            nc.vector.tensor_copy(out=scratch[: t.shape[0], i : i + 1], in_=t[:, 0:1])

        for g in range(PAIRS):
            x_slab = xpool.tile([C, WS, N], F32)
            nc.gpsimd.dma_start(out=x_slab, in_=xs[g])
            xb2 = xbpool.tile([C, WS, N], BF16)
            nc.vector.tensor_copy(out=xb2, in_=x_slab)  # contiguous cast, 2x mode

            for pw in range(WS):
                xw = xb2[:, pw, :]  # [128, 1024] bf16, contiguous
                xw_f32 = x_slab[:, pw, :]  # [128, 1024] f32, contiguous

                # ---- q/k projections: [32, 1024] = W^T.T @ x_win ----
                pqk = ps_m.tile([C, N], F32, tag="mm")
                for h in range(2):
                    nc.tensor.matmul(
                        pqk[:D, bass.ts(h, 512)], wq_sb, xw[:, bass.ts(h, 512)]
                    )
                q_sb = qkpool.tile([D, N], BF16, tag="q")
                nc.vector.tensor_scalar_add(out=q_sb, in0=pqk[:D, :], scalar1=bq_sb)
                pqk2 = ps_m.tile([C, N], F32, tag="mm")
                for h in range(2):
                    nc.tensor.matmul(
                        pqk2[:D, bass.ts(h, 512)], wk_sb, xw[:, bass.ts(h, 512)]
                    )
                k_sb = qkpool.tile([D, N], BF16, tag="k")
                nc.vector.tensor_scalar_add(out=k_sb, in0=pqk2[:D, :], scalar1=bk_sb)

                # ---- v^T[c_out, m] = Wv @ x_win (1 stationary, 2 wide mms),
                # then v[m, c] chunks via DMA XBAR instead of 8 PE matmuls ----
                pv = ps_m.tile([C, N], F32, tag="mm")
                for h in range(2):
                    nc.tensor.matmul(
                        pv[:, bass.ts(h, 512)], wv_sb, xw[:, bass.ts(h, 512)]
                    )
                v_sb = vpool.tile([C, N], BF16)
                nc.vector.tensor_copy(out=v_sb, in_=pv)
                # vt[p, mc, j] = v_sb[j, mc*128+p] = v[m=mc*128+p, c=j]
                vt = vpool.tile([128, NCH, 128], BF16, tag="vt", bufs=3)
                nc.sync.dma_start(out=vt, in_=v_sb, transpose=True)

                # ---- scores + softmax + attn^T + AV, pipelined per half ----
                e_sb = epool.tile([128, NCH, N], BF16)
                z = zpool.tile([128, NCH], F32, tag="z")
                izg = zpool.tile([128, NCH], F32, tag="izg")
                po = ps_o.tile([C, N], F32)
                for hh in range(2):
                    for ncc in range(hh * 4, hh * 4 + 4):
                        ps = ps_s.tile([128, N], F32)
                        for h in range(2):
                            nc.tensor.matmul(
                                ps[:, bass.ts(h, 512)],
                                q_sb[:, bass.ts(ncc, 128)],
                                k_sb[:, bass.ts(h, 512)],
                            )
                        nc.scalar.activation(
                            out=e_sb[:, ncc, :],
                            in_=ps,
                            func=mybir.ActivationFunctionType.Exp,
                            accum_out=z[:, ncc : ncc + 1],
                        )
                    hs = slice(hh * 4, hh * 4 + 4)
                    nc.vector.reciprocal(out=izg[:, hs], in_=z[:, hs])
                    nc.vector.tensor_scalar_mul(
                        out=izg[:, hs], in0=izg[:, hs], scalar1=gam_sb
                    )
                    for ncc in range(hh * 4, hh * 4 + 4):
                        nc.vector.tensor_scalar_mul(
                            out=e_sb[:, ncc, :],
                            in0=e_sb[:, ncc, :],
                            scalar1=izg[:, ncc : ncc + 1],
                        )
                    # ---- attn^T for this n-half via one DMA XBAR transpose ----
                    # in [128, 4096]: f = nccL*1024 + m; out[p, d1, j] =
                    # in[j, d1*128+p], d1 = nccL*8+mc -> at_h[p, nccL*8+mc, j]
                    # = attn[(hh*4+nccL)*128+j, mc*128+p]. Separate tile per
                    # half so AV(h) only depends on its own transpose.
                    at_h = apool.tile([128, NCH * 4, 128], BF16, tag=f"at{hh}", bufs=3)
                    nc.sync.dma_start(
                        out=at_h,
                        in_=e_sb[:, hs, :].rearrange("p a m -> p (a m)"),
                        transpose=True,
                    )
                    at_r = at_h.rearrange("p (a b) j -> p b a j", b=NCH)

                    # ---- out^T[c, n-half] = sum_m v[m, c] * attn^T[m, n] ----
                    for mc in range(NCH):
                        nc.tensor.matmul(
                            po[:, bass.ts(hh, 512)],
                            vt[:, mc, :],
                            at_r[:, mc, :, :],
                            start=(mc == 0),
                            stop=(mc == NCH - 1),
                        )

                # ---- epilogue: y = out^T + gamma*bv + x (in-place into slab) ----
                nc.vector.scalar_tensor_tensor(
                    out=xw_f32,
                    in0=po,
                    scalar=gv_sb,
                    in1=xw_f32,
                    op0=mybir.AluOpType.add,
                    op1=mybir.AluOpType.add,
                )

            nc.gpsimd.dma_start(out=ys[g], in_=x_slab)
    return nc


def _get_nc():
    if "nc" not in _CACHE:
        nc = bacc.Bacc(
            "TRN2",
            target_bir_lowering=False,
            debug=False,
            enable_asserts=False,
            num_devices=NCORES,
        )
        _emit(nc)
        # bacc passes: split multi-sem waits into EventSemaphores (HW allows
        # one wait per instruction), move matmul waits to ldweights, etc.
        nc.finalize()
        _CACHE["nc"] = nc
    return _CACHE["nc"]


def _shard_inputs(x, Wq, bq, Wk, bk, Wv, bv, gamma):
    bf = ml_dtypes.bfloat16
    x = np.ascontiguousarray(np.asarray(x, np.float32))
    wq_h = np.ascontiguousarray(np.asarray(Wq, np.float32).T).astype(bf)
    wk_h = np.ascontiguousarray(np.asarray(Wk, np.float32).T).astype(bf)
    wv_h = np.ascontiguousarray(np.asarray(Wv, np.float32).T).astype(bf)
    bq_h = np.ascontiguousarray(np.asarray(bq, np.float32).reshape(D, 1))
    bk_h = np.ascontiguousarray(np.asarray(bk, np.float32).reshape(D, 1))
    g = float(np.asarray(gamma, np.float32).reshape(-1)[0])
    gv_h = np.ascontiguousarray((g * np.asarray(bv, np.float32)).reshape(C, 1))
    gam_h = np.full((C, 1), g, np.float32)
    # window-major permute: x6[b, c, i, ph, j, pw] -> slab[c, pw, i*32+j]
    x6 = x.reshape(B, C, NH, WS, NW, WS)
    in_maps = []
    for core in range(NCORES):
        slabs = np.stack(
            [
                np.ascontiguousarray(
                    x6[(PAIRS * core + j) // WS, :, :, (PAIRS * core + j) % WS, :, :]
                    .transpose(0, 3, 1, 2)  # [c, pw, i, j]
                    .reshape(C, WS, N)
                )
                for j in range(PAIRS)
            ]
        )
        in_maps.append(
            dict(
                xs=slabs,
                wq=wq_h,
                wk=wk_h,
                wv=wv_h,
                bq=bq_h,
                bk=bk_h,
                gv=gv_h,
                gam=gam_h,
            )
        )
    return in_maps


def kernel(x, Wq, bq, Wk, bk, Wv, bv, gamma):
    nc = _get_nc()
    in_maps = _shard_inputs(x, Wq, bq, Wk, bk, Wv, bv, gamma)
    res = bass_utils.run_bass_kernel_spmd(
        nc, in_maps, core_ids=list(range(NCORES)), trace=TRACE
    )
    LAST["exec_time_ns"] = res.exec_time_ns
    LAST["results"] = res
    y = np.empty((B, C, H, W), np.float32)
    y6 = y.reshape(B, C, NH, WS, NW, WS)
    for core in range(NCORES):
        out = res.results[core]["ys"]  # [PAIRS, C, WS, N]
        for j in range(PAIRS):
            p = PAIRS * core + j
            # [c, pw, i, j] -> [c, i, j, pw]
            y6[p // WS, :, :, p % WS, :, :] = (
                out[j].reshape(C, WS, NH, NW).transpose(0, 2, 3, 1)
            )
    return y



# revision 10
# speedup vs baseline: 1.3211x; 1.3211x over previous
"""ChessBoardAttention Trainium2 kernel (v2: fp8 DoubleRow AV + exact-score restructure).

Math (per chessboard window):
  x: [B=2, C=128, H=256, W=256] f32.  WS=8 phases -> 128 windows of N=1024
  tokens, C=128 channels, D=32 q/k dims.
  S[n,m] = (Wq x_n + bq)·(Wk x_m + bk)
         = x_n^T (Wq^T Wk) x_m + (Wk^T bq)·x_m + [per-n terms, softmax-invariant]
  With yhat_n = (Wq^T Wk)^T x_n + Wk^T bq:  softmax(S) == softmax(yhat^T X)
  exactly.  out = softmax(S) V ; y = gamma*out + x.

Per-window pipeline (16 windows/core, 2 slabs of 8, all windows independent):
  yhat = mh.T @ X (+wc on gpsimd)     (PE)            [128, 1024] bf16
  vt   = X_sel.T @ wv'                (PE, stride-2 column stationaries) fp8
         with wv' = Wv^T * (gamma*zc); m-mapping m = 2*(128*d4 + p) + kt
  S    = yhat_chunk.T @ X             (PE bf16, 16 x 512-row matmuls)
  e    = Exp(S) -> fp8                (ACT, raw unnormalized exp, O(1) values)
  e^T  = XBAR transpose of e viewed as uint16 PAIRS (adjacent fp8 m-pairs ride
         the 2-byte transpose and become DoubleRow k-tiles)
  po   = vt.T @ e^T                   (PE fp8 DoubleRow, 0.5 cyc/row) [c, n]
  zrow = zc_mat.T @ e^T               (PE fp8 DoubleRow) [c, n] = zc*Z[n] (all rows)
  sb   = 1/zrow                       (DVE recip, psum -> bf16)
  y    = (po*sb + gv) + x             (DVE, in-place into the bf16 slab)
Normalization algebra: po*sb = (1/zc)*attn@v' and v' = (Wv x)*(gamma*zc), so
the result is exactly gamma*attn@(Wv x); gv = gamma*bv is exact (attn rows
sum to 1).  x/y ride bf16 slabs (~2e-3 relmax vs 2e-2 tolerance).

Scheduling: the back half (AV onward) of window w is emitted during window
w+2, so PE never waits on exp/transpose latency; PE stays >90% busy and the
HAM clock gate holds 2.4 GHz.  Per-window engine budget: ACT 8.3us (bottleneck),
PE ~7.8us, DVE ~3.4us, GPSIMD ~3.6us, SP(transposes) ~4.9us.
"""

import sys

if "/opt/trn_rl_repo" not in sys.path:
    sys.path.insert(0, "/opt/trn_rl_repo")

from contextlib import ExitStack

import ml_dtypes
import numpy as np

import concourse.bacc as bacc
import concourse.bass as bass
import concourse.mybir as mybir
from concourse import bass_utils
from concourse.tile import TileContext

B, C, H, W = 2, 128, 256, 256
WS = 8
NH, NW = H // WS, W // WS  # 32, 32
N = NH * NW  # 1024 tokens per window
NCORES = 8
PAIRS = 2  # (b, ph) row-groups per core
NWIN = PAIRS * WS  # 16 windows per core
F32 = mybir.dt.float32
BF16 = mybir.dt.bfloat16
F8 = mybir.dt.float8e4
U16 = mybir.dt.uint16
DR = mybir.MatmulPerfMode.DoubleRow
ADD = mybir.AluOpType.add
MULT = mybir.AluOpType.mult

TRACE = False
LAST = {}

_CACHE = {}


def _emit(nc: bass.Bass):
    xs = nc.dram_tensor("xs", [PAIRS, C, WS, N], BF16, kind="ExternalInput").ap()
    mh = nc.dram_tensor("mh", [C, C], BF16, kind="ExternalInput").ap()  # Wq^T Wk
    wv = nc.dram_tensor("wv", [C, C], BF16, kind="ExternalInput").ap()  # scaled Wv^T
    wc = nc.dram_tensor("wc", [C, 1], F32, kind="ExternalInput").ap()  # Wk^T bq
    gv = nc.dram_tensor("gv", [C, 1], F32, kind="ExternalInput").ap()  # gamma*bv
    zc = nc.dram_tensor("zc", [C, 2, C], F8, kind="ExternalInput").ap()  # ~1/gamma
    ys = nc.dram_tensor("ys", [PAIRS, C, WS, N], BF16, kind="ExternalOutput").ap()

    with ExitStack() as ctx:
        tc = ctx.enter_context(TileContext(nc))
        consts = ctx.enter_context(tc.tile_pool(name="consts", bufs=1))
        xpool = ctx.enter_context(tc.tile_pool(name="xpool", bufs=2))
        ypool = ctx.enter_context(tc.tile_pool(name="ypool", bufs=2))
        vpool = ctx.enter_context(tc.tile_pool(name="vpool", bufs=3))
        epool = ctx.enter_context(tc.tile_pool(name="epool", bufs=2))
        apool = ctx.enter_context(tc.tile_pool(name="apool", bufs=3))
        tpool = ctx.enter_context(tc.tile_pool(name="tpool", bufs=2))
        ps_s = ctx.enter_context(tc.tile_pool(name="ps_s", bufs=2, space="PSUM"))
        ps_o = ctx.enter_context(tc.tile_pool(name="ps_o", bufs=1, space="PSUM"))
        ps_x = ctx.enter_context(tc.tile_pool(name="ps_x", bufs=2, space="PSUM"))

        mh_sb = consts.tile([C, C], BF16)
        nc.sync.dma_start(out=mh_sb, in_=mh)
        wv_sb = consts.tile([C, C], BF16)
        nc.sync.dma_start(out=wv_sb, in_=wv)
        wc_sb = consts.tile([C, 1], F32)
        nc.sync.dma_start(out=wc_sb, in_=wc)
        gv_sb = consts.tile([C, 1], F32)
        nc.sync.dma_start(out=gv_sb, in_=gv)
        zc_sb = consts.tile([C, 2, C], F8)
        nc.sync.dma_start(out=zc_sb, in_=zc)

        # Touch every const once so later ops never carry const-DMA waits
        # (walrus allows only one sync wait on TensorScalarPtr).
        scratch = consts.tile([C, 8], F32)
        for i, t in enumerate([mh_sb, wv_sb, wc_sb, gv_sb]):
            nc.vector.tensor_copy(out=scratch[: t.shape[0], i : i + 1], in_=t[:, 0:1])
        with nc.allow_low_precision(reason="const touch"):
            nc.gpsimd.tensor_copy(out=scratch[:, 5:6], in_=zc_sb[:, 0, 0:1])

        x_sb = []
        for g in range(PAIRS):
            slab = xpool.tile([C, WS, N], BF16, tag=f"x{g}", name=f"slab{g}")
            x_sb.append(slab)
            nc.gpsimd.dma_start(out=slab, in_=xs[g])

        st = {}  # w -> dict(at=[h0,h1], vt=..., xw=..., po=..., srow=...)

        def back_av(w):
            """AV + Z matmuls for window w (at-tiles are 2 windows old)."""
            s = st[w]
            # fp8 views r[p, kt, d4, nccL, j]; m = 2*(128*d4+p)+kt,
            # n(half) = nccL*128 + j
            s["r"] = [
                a.bitcast(F8).rearrange("p (a d) (j k) -> p k d a j", d=4, k=2)
                for a in s["at"]
            ]
            po = [ps_o.tile([C, 512], F32, tag=f"po{h}", name=f"po{h}_{w}") for h in range(2)]
            for d4 in range(4):
                for hh in range(2):
                    nc.tensor.matmul(
                        po[hh],
                        s["vt"][:, d4],
                        s["r"][hh][:, :, d4],
                        start=(d4 == 0),
                        stop=(d4 == 3),
                        perf_mode=DR,
                    )
            s["po"] = po
            zp = [ps_x.tile([C, 512], F32, tag="px", name=f"zp{h}_{w}") for h in range(2)]
            for hh in range(2):
                for d4 in range(4):
                    nc.tensor.matmul(
                        zp[hh],
                        zc_sb,
                        s["r"][hh][:, :, d4],
                        start=(d4 == 0),
                        stop=(d4 == 3),
                        perf_mode=DR,
                    )
            sbb = []
            with nc.allow_low_precision(reason="norm row"):
                for hh in range(2):
                    sb_h = tpool.tile([C, 512], BF16, tag=f"sbb{hh}", name=f"sbb{hh}_{w}")
                    nc.vector.reciprocal(out=sb_h, in_=zp[hh])
                    sbb.append(sb_h)
            s["sbb"] = sbb

        def head(w):
            """yhat + v' for window w."""
            g, pw = w // WS, w % WS
            xw = x_sb[g][:, pw, :]  # [128, 1024] bf16
            xr = xw.rearrange("p (j k) -> p k j", k=2)  # stride-2 col view

            y_sb = ypool.tile([C, N], BF16, tag="yhat")
            for h in range(2):
                pY = ps_x.tile([C, 512], F32, tag="px")
                nc.tensor.matmul(pY, mh_sb, xw[:, bass.ts(h, 512)])
                with nc.allow_low_precision(reason="bf16 scores"):
                    nc.vector.tensor_scalar_add(
                        out=y_sb[:, bass.ts(h, 512)], in0=pY, scalar1=wc_sb
                    )

            vt = vpool.tile([C, 4, 2, C], F8, tag="vt")
            for dd in range(2):
                pV = ps_x.tile([C, 512], F32, tag="px")
                for q in range(2):
                    for kt in range(2):
                        nc.tensor.matmul(
                            pV[:, bass.ts(q * 2 + kt, 128)],
                            xr[:, kt, bass.ts(dd * 2 + q, 128)],
                            wv_sb,
                        )
                with nc.allow_low_precision(reason="fp8 v"):
                    nc.vector.tensor_copy(
                        out=vt[:, dd * 2 : dd * 2 + 2],
                        in_=pV.rearrange("p (a b c) -> p a b c", a=2, b=2),
                    )
            st[w] = {"vt": vt, "xw": xw, "y_sb": y_sb}

        def back_norm(w):
            """Broadcast normalization + epilogue for window w."""
            s = st.pop(w)
            for hh in range(2):
                th = tpool.tile([C, 512], BF16, tag="th")
                with nc.allow_low_precision(reason="epilogue"):
                    nc.vector.tensor_tensor(
                        out=th, in0=s["po"][hh], in1=s["sbb"][hh], op=MULT
                    )
                    nc.vector.scalar_tensor_tensor(
                        out=s["xw"][:, bass.ts(hh, 512)],
                        in0=th,
                        scalar=gv_sb,
                        in1=s["xw"][:, bass.ts(hh, 512)],
                        op0=ADD,
                        op1=ADD,
                    )

        def scores(w):
            """S + exp + transposes for window w."""
            s = st[w]
            e_sb = epool.tile([C, WS, N], F8, tag="e")
            e16 = e_sb.bitcast(U16)  # [128, 8, 512]
            at = []
            for hh in range(2):
                for ncc in range(hh * 4, hh * 4 + 4):
                    ps = ps_s.tile([C, N], F32)
                    for h in range(2):
                        nc.tensor.matmul(
                            ps[:, bass.ts(h, 512)],
                            s["y_sb"][:, bass.ts(ncc, 128)],
                            s["xw"][:, bass.ts(h, 512)],
                        )
                    nc.scalar.activation(
                        out=e_sb[:, ncc, :],
                        in_=ps,
                        func=mybir.ActivationFunctionType.Exp,
                    )
                a = apool.tile([C, 16, C], U16, tag=f"at{hh}")
                nc.sync.dma_start(
                    out=a,
                    in_=e16[:, hh * 4 : hh * 4 + 4, :].rearrange("p a m -> p (a m)"),
                    transpose=True,
                )
                at.append(a)
            s["at"] = at

        # Software pipeline, back half lagged 2 windows.
        for w in range(NWIN + 2):
            if w >= 2:
                back_av(w - 2)
            if w < NWIN:
                head(w)
            if w >= 2:
                back_norm(w - 2)
                if (w - 2) % WS == WS - 1:
                    g = (w - 2) // WS
                    nc.gpsimd.dma_start(out=ys[g], in_=x_sb[g])
            if w < NWIN:
                scores(w)
    return nc


def _get_nc():
    if "nc" not in _CACHE:
        nc = bacc.Bacc(
            "TRN2",
            target_bir_lowering=False,
            debug=False,
            enable_asserts=False,
            num_devices=NCORES,
        )
        _emit(nc)
        nc.finalize()
        _CACHE["nc"] = nc
    return _CACHE["nc"]


def _shard_inputs(x, Wq, bq, Wk, bk, Wv, bv, gamma):
    bf = ml_dtypes.bfloat16
    f8 = ml_dtypes.float8_e4m3fn
    x = np.asarray(x, np.float32)
    Wq = np.asarray(Wq, np.float32)
    Wk = np.asarray(Wk, np.float32)
    Wv = np.asarray(Wv, np.float32)
    bq = np.asarray(bq, np.float32)
    bv = np.asarray(bv, np.float32)
    g = float(np.asarray(gamma, np.float32).reshape(-1)[0])

    mh_h = np.ascontiguousarray(Wq.T @ Wk).astype(bf)  # [128, 128]
    wc_h = np.ascontiguousarray((Wk.T @ bq).reshape(C, 1))  # [128, 1]
    gv_h = np.ascontiguousarray((g * bv).reshape(C, 1))
    # zc: fp8 constant ~ 1/gamma; its exact fp8 value is compensated through
    # the wv scale, so the normalization algebra is exact.
    zc_val = np.clip(1.0 / g if g != 0 else 448.0, -448.0, 448.0)
    zc_f8 = np.array(zc_val, np.float32).astype(f8)
    zc_h = np.broadcast_to(zc_f8, (C, 2, C)).copy()
    wv_h = np.ascontiguousarray(Wv.T * (g * float(zc_f8))).astype(bf)

    # window-major permute: x6[b, c, i, ph, j, pw] -> slab[c, pw, i*32+j]
    x6 = x.reshape(B, C, NH, WS, NW, WS)
    in_maps = []
    for core in range(NCORES):
        slabs = np.stack(
            [
                x6[(PAIRS * core + j) // WS, :, :, (PAIRS * core + j) % WS, :, :]
                .transpose(0, 3, 1, 2)  # [c, pw, i, j]
                .reshape(C, WS, N)
                .astype(bf)
                for j in range(PAIRS)
            ]
        )
        in_maps.append(
            dict(xs=slabs, mh=mh_h, wv=wv_h, wc=wc_h, gv=gv_h, zc=zc_h)
        )
    return in_maps


def kernel(x, Wq, bq, Wk, bk, Wv, bv, gamma):
    nc = _get_nc()
    in_maps = _shard_inputs(x, Wq, bq, Wk, bk, Wv, bv, gamma)
    res = bass_utils.run_bass_kernel_spmd(
        nc, in_maps, core_ids=list(range(NCORES)), trace=TRACE
    )
    LAST["exec_time_ns"] = res.exec_time_ns
    LAST["results"] = res
    y = np.empty((B, C, H, W), np.float32)
    y6 = y.reshape(B, C, NH, WS, NW, WS)
    for core in range(NCORES):
        out = res.results[core]["ys"]  # [PAIRS, C, WS, N] bf16
        for j in range(PAIRS):
            p = PAIRS * core + j
            y6[p // WS, :, :, p % WS, :, :] = (
                np.asarray(out[j], np.float32)
                .reshape(C, WS, NH, NW)
                .transpose(0, 2, 3, 1)
            )
    return y
